# revision 43
# baseline (speedup 1.0000x reference)
"""Local (7x7 window) attention kernel for Trainium2, 8 NeuronCores.

Problem: x[8,128,64,64]; q/k/v = 1x1-conv projections of x; attention over the
7x7 spatial neighborhood (zero-padded) summed over channels; softmax over the
49 window positions; y = attn-weighted sum of v over the window.

Sharding: data-parallel over batch B=8 -> one batch element per core.

v3: the fp32 baseline was PE-bound (fp32 matmul = 4 cycles/col).  bf16
everywhere is 4x faster but scores lose too much precision (softmax amplifies
the ~0.1 abs score error into ~4e-2 output error).  So:

  - q/k projections and scores run in fp32r, which is full rate (1 cycle/col)
    when the moving dim is >= 256: scores process QUERY PAIRS (4 rows, 256
    queries) against a 10-row key halo (5 chunks of 128 keys).
  - The AV side is bf16 (em in [0, e^28] after the -40 exp shift, value error
    ~0.4% -> ~3e-3 on y): per 2-row block, 4 matmuls with the em chunk
    stationary and vt [128,129] moving; the 129th all-ones vt column makes
    col 128 of the PSUM the softmax denominator for free.
  - Masking is multiplicative AFTER exp (bf16 0/1 mask; DVE runs all-bf16
    SBUF tensor ops at 2x) with a per-pair knob to run some pairs as
    PE-side additive -1e9 mask matmuls instead, to balance DVE vs PE.
    exp gets bias=-40 so unmasked out-of-window scores (sigma~11) can never
    overflow bf16; the uniform e^-40 factor cancels in numerator/denominator.
  - Normalization happens on the HOST: the kernel returns the [128, 129]
    numerator+denominator blocks (Pool copies PSUM->SBUF, 3 blocks per DMA)
    and the host divides (and adds the dx out-of-image-W correction, scaled
    by e^-40).
  - v = Wv x + bv is computed bf16 (Pool casts x to bf16, Pool adds the bias
    broadcast); q/k biases are DVE adds (f32).
  - Projection groups are interleaved into the attention pair loop so the PE
    does not sit idle during the initial x DMA.
"""

import sys

if "/opt/trn_rl_repo" not in sys.path:
    sys.path.insert(0, "/opt/trn_rl_repo")

import numpy as np

import concourse.bass as bass
import concourse.bacc as bacc
import concourse.mybir as mybir
from concourse import tile
from concourse.bass_utils import run_bass_kernel_spmd

F32 = mybir.dt.float32
F32R = mybir.dt.float32r
BF16 = mybir.dt.bfloat16
NPBF16 = mybir.dt.np(BF16)

B, C, H, W = 8, 128, 64, 64
KW = 7
PAD = KW // 2            # 3
HP = H + 2 * PAD         # 70 padded rows
NPIX = H * W             # 4096
NPPIX = HP * W           # 4480
RPB = 2                  # query rows per block
NBLK = H // RPB          # 32 blocks
NCHUNK = 4               # key chunks per block (AV)
NPAIR = NBLK // 2        # 16 score pairs (4 query rows each)
PCHUNK = 5               # key chunks per pair (10 rows)
NVC = NPPIX // 128       # 35 vt chunks
VST = 129                # vt chunk stride (128 values + 1 ones column)
GRP = 3                  # blocks per output batch (129*3=387 <= 512 PSUM)
ESHIFT = -40.0           # exp bias: em = exp(s - 40), cancels in num/den
# pairs whose mask runs as PE additive matmuls instead of DVE multiplies
# (pair 0 must be a DVE pair: the mask constants land after its scores run)
PE_MASK_PAIRS = frozenset(range(1, NPAIR - 1, 2))
# v chunks processed per PSUM tile / DVE bias-add: (first chunk, count,
# partition range) — edge chunks 1 and 33 are half-height and go alone so
# their garbage halves never overwrite the zero padding
VGROUPS = (
    (1, 1, (64, 128)),
    *[(2 + 4 * g, 4, (0, 128)) for g in range(7)],
    (30, 3, (0, 128)),
    (33, 1, (0, 64)),
)

_CACHE = {}


def _build_mask01():
    """mask01[p, 256*j + qi]: 1 if key (pair-chunk j, within-chunk p) is in
    the 7x7 window of pair-query qi (4 rows x 64), else 0. Pair-independent."""
    m = np.zeros((128, PCHUNK * 256), dtype=np.float32)
    for j in range(PCHUNK):
        for p in range(128):
            r, wk = p // 64, p % 64
            for qi in range(256):
                rq, wq = qi // 64, qi % 64
                dh = 2 * j + r - 3 - rq
                if abs(dh) <= PAD and abs(wk - wq) <= PAD:
                    m[p, 256 * j + qi] = 1.0
    return m


def _build_dx():
    """#window positions outside the image in W, per query pixel of a block
    (2 rows x 64): 7 window rows x missing w columns."""
    dx = np.zeros((128,), dtype=np.float32)
    for qi in range(128):
        wq = qi % 64
        dx[qi] = float(KW * (max(0, PAD - wq) + max(0, wq - (W - 1 - PAD))))
    return dx


def _vchunk_groups():
    """vt chunk j (j=1..33) reads x rows (2j-3, 2j-2) -> x/projection chunk
    m = last chunk whose 8-row range covers those rows."""
    groups = {m: [] for m in range(8)}
    for j in range(1, NVC - 1):
        if j == 1:
            need = 0
        elif j == NVC - 2:
            need = ((H - 1)) // 8
        else:
            need = (2 * j - 2) // 8
        groups[need].append(j)
    return groups


def _build_bass():
    nc = bacc.Bacc()

    # the q/k scores path runs in fp32r (full PE rate at >=256 moving cols);
    # every producer feeding an fp32r matmul must itself write fp32r
    x_d = nc.dram_tensor("x", [C, NPIX], F32R, kind="ExternalInput")
    # consts combined into one tensor per dtype so each lands in ONE DMA:
    # cwr: wqt | wkt (fp32r); cf32: bq | bk | bvb; cbf16: wvt | ident |
    # mask01 | maskneg
    cwr_d = nc.dram_tensor("cwr", [C, 256], F32R, kind="ExternalInput")
    cf32_d = nc.dram_tensor("cf32", [C, 514], F32, kind="ExternalInput")
    cbf16_d = nc.dram_tensor(
        "cbf16", [128, 256 + 2 * PCHUNK * 256], BF16, kind="ExternalInput"
    )
    y_d = nc.dram_tensor("y", [128, NBLK * VST], F32, kind="ExternalOutput")

    vgroups = _vchunk_groups()

    with tile.TileContext(nc) as tc:
        with (
            tc.tile_pool(name="const", bufs=1) as cpool,
            tc.tile_pool(name="big", bufs=1) as bigpool,
            tc.tile_pool(name="sb_er", bufs=3) as sb_er,
            tc.tile_pool(name="sb_em", bufs=3) as sb_em,
            tc.tile_pool(name="sb_y", bufs=4) as sb_y,
        ):
            # ---- constants / persistent tensors ----
            cwr = cpool.tile([C, 256], F32R)
            cf32 = cpool.tile([C, 514], F32)
            cbf16 = cpool.tile([128, 256 + 2 * PCHUNK * 256], BF16)
            wqt = cwr[:, 0:128]
            wkt = cwr[:, 128:256]
            bq = cf32[:, 0:1]
            bk = cf32[:, 1:2]
            bvbg = cf32[:, 2:514]
            wvt = cbf16[:, 0:128]
            ident = cbf16[:, 128:256]
            mask01 = cbf16[:, 256 : 256 + PCHUNK * 256]
            maskneg = cbf16[:, 256 + PCHUNK * 256 : 256 + 2 * PCHUNK * 256]
            warm = cpool.tile([128, 1], F32)
            eshift = cpool.tile([128, 1], F32)

            x_s = bigpool.tile([C, NPIX], F32R)
            xb_s = bigpool.tile([C, NPIX], BF16)
            q_s = bigpool.tile([C, NPIX], F32R)
            kp_s = bigpool.tile([C, NPPIX], F32R)
            vt_s = bigpool.tile([128, NVC * VST], BF16)

            # DGE queues: ACT carries cf32 (weights/biases, needed first) and
            # the last three x chunks, then is free for the q-bias adds; SP
            # carries x0..x4 and then the bulky mask/wvt constants (wvt is
            # needed by the v matmuls ~5us in, masks by the pair-0 DVE mask
            # ~7us in).
            # weights first on the Pool queue so the first projection matmul
            # is not gated by the ACT table preload; ACT only does the q-bias
            # adds before the exps.
            nc.gpsimd.dma_start(cwr[:], cwr_d[:])
            nc.scalar.dma_start(cf32[:], cf32_d[:])
            for m in (5, 6, 7):
                sl = slice(512 * m, 512 * (m + 1))
                nc.gpsimd.dma_start(x_s[:, sl], x_d[:, sl])
            for m in range(5):
                sl = slice(512 * m, 512 * (m + 1))
                nc.sync.dma_start(x_s[:, sl], x_d[:, sl])
            nc.sync.dma_start(cbf16[:], cbf16_d[:])

            # preload the ACT exp table while DMAs run
            nc.gpsimd.memset(warm[:], 0.0)
            nc.gpsimd.memset(eshift[:], ESHIFT)
            nc.scalar.activation(warm[:], warm[:], mybir.ActivationFunctionType.Exp)

            # zero-padding of kp and vt; ones columns of vt.  Memset cannot
            # write fp32r, but DVE can: stage zeros in f32 and copy.
            zpad = cpool.tile([128, PAD * W], F32)
            nc.gpsimd.memset(zpad[:], 0.0)
            nc.vector.tensor_copy(kp_s[:, 0 : PAD * W], zpad[:])
            nc.vector.tensor_copy(kp_s[:, (PAD + H) * W : NPPIX], zpad[:])
            nc.gpsimd.memset(vt_s[:, 0 : 2 * VST], 0.0)
            nc.gpsimd.memset(vt_s[:, (NVC - 2) * VST : NVC * VST], 0.0)
            nc.gpsimd.memset(vt_s[:, 128 : NVC * VST : VST], 1.0)

            # xb casts on Pool (its only SBUF->SBUF job; GPSIMD cannot
            # touch PSUM on real hardware)
            for m in range(8):
                sl = slice(512 * m, 512 * (m + 1))
                nc.gpsimd.tensor_copy(xb_s[:, sl], x_s[:, sl].bitcast(F32))

            # ---- projections (own PSUM pools, released before the big
            # attention score tiles are allocated; 3+3+2 banks) ----
            with (
                tc.tile_pool(name="ps_k", bufs=3, space="PSUM") as ps_k,
                tc.tile_pool(name="ps_q", bufs=3, space="PSUM") as ps_q,
                tc.tile_pool(name="ps_v", bufs=2, space="PSUM") as ps_v,
            ):
                for m in range(8):
                    sl = slice(512 * m, 512 * (m + 1))
                    ksl = slice(PAD * W + 512 * m, PAD * W + 512 * (m + 1))
                    pk = ps_k.tile([128, 512], F32, tag="pk", name=f"pk{m}")
                    nc.tensor.matmul(
                        pk[:],
                        wkt[:],
                        x_s[:, sl],
                        start=True,
                        stop=True,
                    )
                    nc.vector.tensor_scalar_add(kp_s[:, ksl], pk[:], bk[:])
                    pq = ps_q.tile([128, 512], F32, tag="pq", name=f"pq{m}")
                    nc.tensor.matmul(
                        pq[:],
                        wqt[:],
                        x_s[:, sl],
                        start=True,
                        stop=True,
                    )
                    # q bias on ACT (idle during the prologue) so DVE only
                    # paces the k-bias adds
                    nc.scalar.activation(
                        q_s[:, sl], pq[:],
                        mybir.ActivationFunctionType.Identity, bias=bq[:],
                    )
                # v chunks batched 4-per-PSUM-bank so the bias-add+copy to
                # SBUF (DVE; Pool cannot read PSUM) is ~9 big ops, not 33
                for j0, n, (p0, p1) in VGROUPS:
                    pv = ps_v.tile([128, 512], F32, tag="pv", name=f"pv{j0}")
                    for i in range(n):
                        j = j0 + i
                        if j == 1:
                            lhsT = xb_s[:, 0:64]
                        elif j == NVC - 2:
                            lhsT = xb_s[:, (H - 1) * W : NPIX]
                        else:
                            r0 = 2 * j - 3
                            lhsT = xb_s[:, r0 * W : (r0 + 2) * W]
                        nc.tensor.matmul(
                            pv[p0:p1, 128 * i : 128 * (i + 1)],
                            lhsT, wvt[:],
                            start=True, stop=True,
                        )
                    dst = (
                        vt_s[p0:p1, VST * j0 : VST * (j0 + n)]
                        .rearrange("p (c k) -> p c k", c=n)[:, :, 0:128]
                    )
                    src = pv[p0:p1, 0 : 128 * n].rearrange(
                        "p (c k) -> p c k", c=n
                    )
                    bvs = bvbg[p0:p1, 0 : 128 * n].rearrange(
                        "p (c k) -> p c k", c=n
                    )
                    nc.vector.tensor_add(dst, src, bvs)

            # ---- attention ----
            ps_s = tc.alloc_tile_pool(name="ps_s", bufs=2, space="PSUM")
            ps_av = tc.alloc_tile_pool(name="ps_av", bufs=2, space="PSUM")
            em_tiles = {}
            pav_tiles = {}

            def s_phase(P):
                """Scores for query pair P (4 rows, 256 queries) vs its 10-row
                key halo, fp32r; exp (shifted by -40) to bf16; 0/1 mask."""
                sps = ps_s.tile([128, PCHUNK * 256], F32, tag="sps")
                pe_mask = P in PE_MASK_PAIRS
                for j in range(PCHUNK):
                    sl = slice(256 * j, 256 * (j + 1))
                    if pe_mask:
                        nc.tensor.matmul(
                            sps[:, sl], ident[:], maskneg[:, sl],
                            start=True, stop=False,
                        )
                    kc = 128 * (2 * P + j)
                    nc.tensor.matmul(
                        sps[:, sl],
                        kp_s[:, kc : kc + 128],
                        q_s[:, 256 * P : 256 * (P + 1)],
                        start=not pe_mask,
                        stop=True,
                    )
                er = sb_er.tile([128, PCHUNK * 256], BF16, tag="er")
                nc.scalar.activation(
                    er[:], sps[:], mybir.ActivationFunctionType.Exp, bias=eshift[:]
                )
                if pe_mask:
                    em_tiles[P] = er
                else:
                    em = sb_em.tile([128, PCHUNK * 256], BF16, tag="em")
                    nc.vector.tensor_mul(em[:], er[:], mask01[:])
                    em_tiles[P] = em

            def av_phase(b):
                """AV for 2-row block b: 4 bf16 matmuls, em chunk stationary,
                vt [128,129] moving; col 128 accumulates sum(em)."""
                P, h = b // 2, b % 2
                em = em_tiles[P]
                g, bb = b // GRP, b % GRP
                if bb == 0:
                    nb = min(GRP, NBLK - b)
                    pav_tiles[g] = ps_av.tile(
                        [128, nb * VST], F32, tag="pav", name=f"pav{g}"
                    )
                pav = pav_tiles[g][:, VST * bb : VST * bb + VST]
                for i in range(NCHUNK):
                    # pair chunk j = h+i, query half h
                    esl = slice(256 * (h + i) + 128 * h, 256 * (h + i) + 128 * (h + 1))
                    vc = VST * (b + i)
                    nc.tensor.matmul(
                        pav,
                        em[:, esl],
                        vt_s[:, vc : vc + VST],
                        start=(i == 0),
                        stop=(i == NCHUNK - 1),
                    )
                if h == 1:
                    em_tiles.pop(P)

            def out_phase(g):
                """Copy numerator+denominator PSUM group to SBUF (Pool) and
                DMA out; the host divides."""
                b0 = GRP * g
                nb = min(GRP, NBLK - b0)
                pav = pav_tiles.pop(g)
                ysb = sb_y.tile([128, nb * VST], F32, tag="ysb")
                nc.vector.tensor_copy(ysb[:], pav[:])
                nc.sync.dma_start(
                    y_d[:, VST * b0 : VST * (b0 + nb)], ysb[:]
                )

            # 2-pair lookahead: the s->exp->mask chain latency (~2 pairs of
            # PE work) must be hidden, so AV for pair P runs at loop step P+2.
            LOOKAHEAD = 2
            for P in range(NPAIR + LOOKAHEAD):
                if P < NPAIR:
                    s_phase(P)
                if P >= LOOKAHEAD:
                    for h in range(2):
                        b = 2 * (P - LOOKAHEAD) + h
                        av_phase(b)
                        if b % GRP == GRP - 1 or b == NBLK - 1:
                            out_phase(b // GRP)

            ps_av.release()
            ps_s.release()

    nc.finalize()
    return nc


def get_nc():
    if "nc" not in _CACHE:
        _CACHE["nc"] = _build_bass()
    return _CACHE["nc"]


def prepare_in_maps(x, Wq, bq, Wk, bk, Wv, bv):
    x = np.asarray(x, dtype=np.float32)
    if "mask01" not in _CACHE:
        m = _build_mask01()
        _CACHE["mask01"] = np.ascontiguousarray(m.astype(NPBF16))
        _CACHE["maskneg"] = np.ascontiguousarray(((m - 1.0) * 1e9).astype(NPBF16))
        _CACHE["dx"] = _build_dx()
        _CACHE["ident"] = np.eye(128, dtype=NPBF16)
    cwr = np.concatenate(
        [np.asarray(Wq, np.float32).T, np.asarray(Wk, np.float32).T], axis=1
    )
    cf32 = np.concatenate(
        [
            np.asarray(bq, np.float32).reshape(C, 1),
            np.asarray(bk, np.float32).reshape(C, 1),
            np.tile(np.asarray(bv, np.float32).reshape(1, C), (128, 4)),
        ],
        axis=1,
    )
    cbf16 = np.concatenate(
        [
            np.asarray(Wv, np.float32).T.astype(NPBF16),
            _CACHE["ident"],
            _CACHE["mask01"],
            _CACHE["maskneg"],
        ],
        axis=1,
    )
    common = {
        "cwr": np.ascontiguousarray(cwr),
        "cf32": np.ascontiguousarray(cf32),
        "cbf16": np.ascontiguousarray(cbf16),
    }
    return [
        dict(common, x=np.ascontiguousarray(x[b].reshape(C, NPIX)))
        for b in range(B)
    ]


def gather_output(results):
    # y per core is [128, NBLK*129]: per block b, cols [129b, 129b+128) hold
    # the numerator for pixel 128b+p channel c; col 129b+128 is sum(em).
    dx = _CACHE["dx"] * np.exp(np.float32(ESHIFT))
    ys = []
    for b in range(B):
        yr = np.asarray(results[b]["y"], np.float32).reshape(128, NBLK, VST)
        num = yr[:, :, :128]                  # [p, blk, c]
        den = yr[:, :, 128] + dx[:, None]     # [p, blk]
        y = num / den[:, :, None]
        ys.append(y.transpose(1, 0, 2).reshape(NPIX, C))
    yt = np.stack(ys)                         # [B, NPIX, C]
    return np.ascontiguousarray(yt.transpose(0, 2, 1).reshape(B, C, H, W))


def kernel(x, Wq, bq, Wk, bk, Wv, bv):
    in_maps = prepare_in_maps(x, Wq, bq, Wk, bk, Wv, bv)
    res = run_bass_kernel_spmd(get_nc(), in_maps, list(range(B))).results
    return gather_output(res)


if __name__ == "__main__":
    rng = np.random.default_rng(0)
    xs = rng.standard_normal((B, C, H, W), dtype=np.float32)
    ws = [rng.standard_normal((C, C), dtype=np.float32) / np.sqrt(C) for _ in range(3)]
    bs = [rng.standard_normal(C).astype(np.float32) * 0.01 for _ in range(3)]
    y = kernel(xs, ws[0], bs[0], ws[1], bs[1], ws[2], bs[2])
    print(y.shape, y.dtype)
